# revision 12
# baseline (speedup 1.0000x reference)
"""CharRNN (2-layer LSTM + softmax CE) Trainium2 Bass kernel.

Sharding: data-parallel over batch (B=64 -> 8 rows/core on 8 cores).
Each core runs the full T=128 recurrence for its 8 sequences and the
cross-entropy over its own 1024 tokens; host sums the 8 partial NLLs.

Device layout (per core):
  - Cell math in batch-on-partitions layout: cell0(t) occupies PSUM/SBUF
    partitions 0:8, cell1(t-1) occupies partitions 32:40 (distinct PE
    column-groups so their matmuls can overlap on the array).
  - z = [x,h] @ W computed with x_h^T as the stationary operand (lhsT) and
    W streamed as the moving operand (weights stream at 1 col/cycle; this
    is the cheap direction for tiny batch).
  - h is transposed each step via TensorE transpose; h1^T columns
    accumulate into OUT^T which later feeds the logits matmul as lhsT.
  - CE: logits tile [128 tokens, 500 vocab] per matmul pair, Exp with
    fused accumulate (sum over vocab chunk), no max-shift (logits are
    O(+-6) for this model), target logit via indirect row gather of the
    host-transposed softmax weights and a fused multiply-reduce.
"""

import sys

for _p in ("/opt/trn_rl_repo",):
    if _p not in sys.path:
        sys.path.insert(0, _p)

import numpy as np

import concourse.bass as bass
import concourse.mybir as mybir
import concourse.tile as tile
from concourse.bass_utils import run_bass_kernel_spmd
from concourse.masks import make_identity

F32 = mybir.dt.float32
I32 = mybir.dt.int32
AF = mybir.ActivationFunctionType
ALU = mybir.AluOpType
AX = mybir.AxisListType

# Problem shapes (hardcoded per contest rules).
V, B, T, U = 16000, 64, 128, 256
NCORES = 8
BL = B // NCORES            # 8 batch rows per core
NTOK = BL * T               # 1024 tokens per core
G4 = 4 * U                  # 1024 gate width
NG = 8                      # embedding gathers per core (128 tokens each)
VC = 500                    # vocab chunk for CE
NVC = V // VC               # 32 chunks
NTT = NTOK // 128           # 8 token tiles for CE

# Gate permutation: reference z columns are [i|j|f|o]; we reorder to
# [i|o|f|j] so sigmoid covers one contiguous [0:768] span and tanh(j) the
# tail. Column ranges in permuted space:
_I = slice(0 * U, 1 * U)    # sigmoid (input gate)
_O = slice(1 * U, 2 * U)    # sigmoid (output gate)
_F = slice(2 * U, 3 * U)    # sigmoid (forget gate, +1.0 bias)
_J = slice(3 * U, 4 * U)    # tanh    (cell candidate)
GPERM = np.r_[0:U, 3 * U:4 * U, 2 * U:3 * U, U:2 * U]

FORGET_BIAS = 1.0

_CACHE = {}


def _split_multiwaits(nc):
    """Walrus (CoreV3 codegen) supports only ONE semaphore wait per HW
    instruction (NEURON_ISA_TPB_EVENTS has a single wait slot) and errors
    out on instructions carrying more. Tile emits multi-wait sync_info
    freely, so split: for an instruction with k>1 waits, insert k-1 NoOps
    on the same engine queue immediately before it, each carrying one
    wait. Identical semantics (the queue processes waits in order)."""
    cnt = 0
    for fn in nc.m.functions:
        for b in fn.blocks:
            live = b.instructions
            out = []
            changed = False
            for i in live:
                si = getattr(i, "sync_info", None)
                waits = list(si.on_wait) if si is not None else []
                movable_idx = [
                    k for k, w in enumerate(waits)
                    if getattr(w, "wait_reg", None) is None
                ]
                if len(waits) > 1 and len(movable_idx) >= 1:
                    # keep the last movable wait (plus any reg-waits) on the
                    # original; hoist the rest onto NoOps.
                    hoist = movable_idx[:-1] if len(movable_idx) == len(waits) \
                        else movable_idx
                    hoist_set = set(hoist)
                    if len(waits) - len(hoist_set) > 1:
                        # still >1 left (multiple reg waits?) — hoist all movable
                        hoist_set = set(movable_idx)
                    for k in sorted(hoist_set):
                        nop = mybir.InstNoOp(name=f"I-nopw{cnt}", ins=[], outs=[])
                        cnt += 1
                        nop.engine = i.engine
                        nop.sync_info = mybir.SyncInfo(
                            on_wait=[waits[k]], on_update=[])
                        out.append(nop)
                    keep = [w for k, w in enumerate(waits) if k not in hoist_set]
                    i.sync_info = mybir.SyncInfo(
                        on_wait=keep, on_update=list(si.on_update))
                    changed = True
                out.append(i)
            if changed:
                live.clear()
                live.extend(out)
    return cnt


def _build_program(b0_nonzero: bool, b1_nonzero: bool, smb_nonzero: bool):
    nc = bass.Bass()

    emb = nc.declare_dram_parameter("emb", [V, U], F32, isOutput=False)
    w0 = nc.declare_dram_parameter("w0", [2 * U, G4], F32, isOutput=False)
    w1 = nc.declare_dram_parameter("w1", [2 * U, G4], F32, isOutput=False)
    b0v = nc.declare_dram_parameter("b0v", [1, G4], F32, isOutput=False)
    b1v = nc.declare_dram_parameter("b1v", [1, G4], F32, isOutput=False)
    smw = nc.declare_dram_parameter("smw", [U, V], F32, isOutput=False)
    smb = nc.declare_dram_parameter("smb", [1, V], F32, isOutput=False)
    wvb = nc.declare_dram_parameter("wvb", [V, 264], F32, isOutput=False)
    idx_e = nc.declare_dram_parameter("idx_e", [128, NG], I32, isOutput=False)
    idx_t = nc.declare_dram_parameter("idx_t", [128, NTT], I32, isOutput=False)
    nll_out = nc.declare_dram_parameter("nll_out", [1, 1], F32, isOutput=True)

    with tile.TileContext(nc) as tc:
        with (
            tc.tile_pool(name="const", bufs=1) as cpool,
            tc.tile_pool(name="wpool", bufs=1) as wpool,
            tc.tile_pool(name="persist", bufs=1) as ppool,
        ):
            ident = cpool.tile([128, 128], F32)
            make_identity(nc, ident[:])
            ones_row = cpool.tile([1, 128], F32)
            nc.gpsimd.memset(ones_row[:], 1.0)
            ones_col = cpool.tile([128, 1], F32)
            nc.gpsimd.memset(ones_col[:], 1.0)

            # Weights resident in SBUF as 4 k-tiles each: [128, 4*1024].
            w0t = wpool.tile([128, 4, G4], F32)
            w1t = wpool.tile([128, 4, G4], F32)
            for k in range(4):
                nc.sync.dma_start(w0t[:, k, :], w0[k * 128:(k + 1) * 128, :])
                nc.sync.dma_start(w1t[:, k, :], w1[k * 128:(k + 1) * 128, :])
            b0t = cpool.tile([1, G4], F32)
            b1t = cpool.tile([1, G4], F32)
            nc.sync.dma_start(b0t[:], b0v[:])
            nc.sync.dma_start(b1t[:], b1v[:])
            if smb_nonzero:
                smbt = cpool.tile([1, V], F32)
                nc.sync.dma_start(smbt[:], smb[:])

            idx_et = cpool.tile([128, NG], I32)
            idx_tt = cpool.tile([128, NTT], I32)
            nc.sync.dma_start(idx_et[:], idx_e[:])
            nc.sync.dma_start(idx_tt[:], idx_t[:])

            # OUT^T: h1 transposed, u on partitions (2 half-tiles), token cols.
            outt = ppool.tile([128, 2, NTOK], F32)

            # ---------------- embedding gather + transpose ----------------
            xts = []
            with tc.tile_pool(name="xt", bufs=NG) as xtpool:
                with (
                    tc.tile_pool(name="xg", bufs=3) as xgpool,
                    tc.tile_pool(name="ptx", bufs=2, space=bass.MemorySpace.PSUM) as ptxpool,
                ):
                    for g in range(NG):
                        xg = xgpool.tile([128, U], F32, tag="xg")
                        nc.gpsimd.indirect_dma_start(
                            out=xg[:],
                            out_offset=None,
                            in_=emb[:],
                            in_offset=bass.IndirectOffsetOnAxis(ap=idx_et[:, g:g + 1], axis=0),
                        )
                        xt = xtpool.tile([128, 2, 128], F32, tag="xt")
                        xts.append(xt)
                        for u in range(2):
                            ptx = ptxpool.tile([128, 128], F32, tag="ptx")
                            nc.tensor.matmul(ptx[:], xg[:, u * 128:(u + 1) * 128], ident[:], start=True, stop=True)
                            nc.vector.tensor_copy(xt[:, u, :], ptx[:])

                # ---------------- LSTM recurrence ----------------
                with (
                    tc.tile_pool(name="zpsum", bufs=2, space=bass.MemorySpace.PSUM) as zpool,
                    tc.tile_pool(name="ptp", bufs=2, space=bass.MemorySpace.PSUM) as ptpool,
                    tc.tile_pool(name="gates", bufs=2) as gpool,
                    tc.tile_pool(name="state", bufs=2) as spool,
                    tc.tile_pool(name="hT", bufs=3) as htpool,
                ):
                    c_prev = None
                    h0T_last = None   # h0^T(s-1): [128, 16] (u-halves x batch)

                    def emit_group(mms, tp):
                        for i, (o, l, r) in enumerate(mms):
                            nc.tensor.matmul(
                                o, l, r,
                                start=(i == 0), stop=(i == len(mms) - 1),
                                tile_position=tp)

                    for s in range(T + 1):
                        t0 = s            # cell0 step index
                        t1 = s - 1        # cell1 step index
                        cell0 = t0 < T
                        cell1 = 0 <= t1 < T

                        z = zpool.tile([128, G4], F32, tag="z")

                        if cell0:
                            g = t0 // 16
                            c0 = (t0 % 16) * BL
                            xt = xts[g]
                            for n in range(2):
                                ns = slice(n * 512, (n + 1) * 512)
                                mms = [
                                    (z[0:BL, ns], xt[:, 0, c0:c0 + BL], w0t[:, 0, ns]),
                                    (z[0:BL, ns], xt[:, 1, c0:c0 + BL], w0t[:, 1, ns]),
                                ]
                                if b0_nonzero:
                                    mms.append((z[0:BL, ns], ones_row[0:1, 0:BL], b0t[0:1, ns]))
                                elif n == 1:
                                    bs = slice(512, 768)
                                    mms.append((z[0:BL, bs], ones_row[0:1, 0:BL], b0t[0:1, bs]))
                                if t0 >= 1:
                                    mms.append((z[0:BL, ns], h0T_last[:, 0:BL], w0t[:, 2, ns]))
                                    mms.append((z[0:BL, ns], h0T_last[:, BL:2 * BL], w0t[:, 3, ns]))
                                emit_group(mms, None)

                        if cell1:
                            for n in range(2):
                                ns = slice(n * 512, (n + 1) * 512)
                                mms = [
                                    (z[32:32 + BL, ns], h0T_last[:, 0:BL], w1t[:, 0, ns]),
                                    (z[32:32 + BL, ns], h0T_last[:, BL:2 * BL], w1t[:, 1, ns]),
                                ]
                                if b1_nonzero:
                                    mms.append((z[32:32 + BL, ns], ones_row[0:1, 0:BL], b1t[0:1, ns]))
                                elif n == 1:
                                    bs = slice(512, 768)
                                    mms.append((z[32:32 + BL, bs], ones_row[0:1, 0:BL], b1t[0:1, bs]))
                                if t1 >= 1:
                                    mms.append((z[32:32 + BL, ns],
                                                outt[:, 0, (t1 - 1) * BL:t1 * BL], w1t[:, 2, ns]))
                                    mms.append((z[32:32 + BL, ns],
                                                outt[:, 1, (t1 - 1) * BL:t1 * BL], w1t[:, 3, ns]))
                                emit_group(mms, (0, 32))

                        # partition range for the elementwise phase
                        if cell0 and cell1:
                            lo, hi = 0, 40
                        elif cell0:
                            lo, hi = 0, BL
                        else:
                            lo, hi = 32, 40

                        gt = gpool.tile([128, 3 * U], F32, tag="G")
                        tj = gpool.tile([128, U], F32, tag="tj")
                        nc.scalar.activation(gt[lo:hi, :], z[lo:hi, 0:3 * U], AF.Sigmoid)
                        nc.scalar.activation(tj[lo:hi, :], z[lo:hi, 3 * U:G4], AF.Tanh)

                        c_new = spool.tile([128, U], F32, tag="c")
                        if s == 0:
                            # c_new = sigmoid(i) * tanh(j); zero cell1 rows so the
                            # s=1 packed update sees c1(-1) = 0.
                            nc.vector.tensor_tensor(
                                out=c_new[lo:hi, :], in0=gt[lo:hi, _I], in1=tj[lo:hi, :],
                                op=ALU.mult)
                            nc.gpsimd.memset(c_new[32:40, :], 0.0)
                        else:
                            m1 = gpool.tile([128, U], F32, tag="m1")
                            m2 = gpool.tile([128, U], F32, tag="m2")
                            nc.vector.tensor_tensor(
                                out=m1[lo:hi, :], in0=gt[lo:hi, _F], in1=c_prev[lo:hi, :],
                                op=ALU.mult)
                            nc.vector.tensor_tensor(
                                out=m2[lo:hi, :], in0=gt[lo:hi, _I], in1=tj[lo:hi, :],
                                op=ALU.mult)
                            nc.vector.tensor_tensor(
                                out=c_new[lo:hi, :], in0=m1[lo:hi, :], in1=m2[lo:hi, :],
                                op=ALU.add)

                        tc_t = gpool.tile([128, U], F32, tag="tc")
                        h_t = gpool.tile([128, U], F32, tag="h")
                        nc.scalar.activation(tc_t[lo:hi, :], c_new[lo:hi, :], AF.Tanh)
                        nc.vector.tensor_tensor(
                            out=h_t[lo:hi, :], in0=tc_t[lo:hi, :], in1=gt[lo:hi, _O],
                            op=ALU.mult)

                        h0T_t = None
                        if cell0:
                            h0T_t = htpool.tile([128, 2 * BL], F32, tag="h0T", name="h0T_t")
                        for u in range(2):
                            pt = ptpool.tile([128, 128], F32, tag="pt")
                            nc.tensor.matmul(pt[:], h_t[:, u * 128:(u + 1) * 128], ident[:], start=True, stop=True)
                            if cell0:
                                nc.vector.tensor_copy(h0T_t[:, u * BL:(u + 1) * BL], pt[:, 0:BL])
                            if cell1:
                                nc.vector.tensor_copy(
                                    outt[:, u, t1 * BL:(t1 + 1) * BL], pt[:, 32:40])

                        if cell0:
                            h0T_last = h0T_t
                        c_prev = c_new

            # ---------------- cross-entropy phase ----------------
            with (
                tc.tile_pool(name="ce_w", bufs=3) as cwpool,
                tc.tile_pool(name="ce_ps", bufs=3, space=bass.MemorySpace.PSUM) as lppool,
                tc.tile_pool(name="ce_pt", bufs=2, space=bass.MemorySpace.PSUM) as ptcpool,
                tc.tile_pool(name="ce_sb", bufs=3) as cspool,
                tc.tile_pool(name="ce_acc", bufs=1) as capool,
            ):
                sums = capool.tile([128, NTT, NVC], F32)
                for vc in range(NVC):
                    wt = cwpool.tile([128, 2, VC], F32, tag="wt")
                    for u in range(2):
                        nc.sync.dma_start(
                            wt[:, u, :], smw[u * 128:(u + 1) * 128, vc * VC:(vc + 1) * VC])
                    for tk in range(NTT):
                        lp = lppool.tile([128, VC], F32, tag="lp")
                        nc.tensor.matmul(
                            lp[:], outt[:, 0, tk * 128:(tk + 1) * 128], wt[:, 0, :],
                            start=True, stop=False)
                        nc.tensor.matmul(
                            lp[:], outt[:, 1, tk * 128:(tk + 1) * 128], wt[:, 1, :],
                            start=False, stop=not smb_nonzero)
                        if smb_nonzero:
                            nc.tensor.matmul(
                                lp[:], ones_row[0:1, :], smbt[0:1, vc * VC:(vc + 1) * VC],
                                start=False, stop=True)
                        es = cspool.tile([128, VC], F32, tag="es")
                        nc.scalar.activation(
                            es[:], lp[:], AF.Exp, accum_out=sums[:, tk, vc:vc + 1])

                lns = capool.tile([128, NTT], F32)
                dots = capool.tile([128, NTT], F32)
                bts = capool.tile([128, NTT], F32)
                for tk in range(NTT):
                    s1 = cspool.tile([128, 1], F32, tag="s1")
                    nc.vector.tensor_reduce(
                        out=s1[:], in_=sums[:, tk, :], axis=AX.X, op=ALU.add)
                    nc.scalar.activation(lns[:, tk:tk + 1], s1[:], AF.Ln)

                    wtg = cspool.tile([128, 264], F32, tag="wtg")
                    nc.gpsimd.indirect_dma_start(
                        out=wtg[:],
                        out_offset=None,
                        in_=wvb[:],
                        in_offset=bass.IndirectOffsetOnAxis(ap=idx_tt[:, tk:tk + 1], axis=0),
                    )
                    outb = cspool.tile([128, U], F32, tag="outb")
                    for u in range(2):
                        ptc = ptcpool.tile([128, 128], F32, tag="ptc")
                        nc.tensor.matmul(
                            ptc[:], outt[:, u, tk * 128:(tk + 1) * 128], ident[:],
                            start=True, stop=True)
                        nc.vector.tensor_copy(outb[:, u * 128:(u + 1) * 128], ptc[:])
                    ttr = cspool.tile([128, U], F32, tag="ttr")
                    nc.vector.tensor_tensor(
                        out=ttr[:], in0=outb[:], in1=wtg[:, 0:U], op=ALU.mult)
                    nc.vector.tensor_reduce(
                        out=dots[:, tk:tk + 1], in_=ttr[:], axis=AX.X, op=ALU.add)
                    nc.vector.tensor_copy(bts[:, tk:tk + 1], wtg[:, U:U + 1])

                # nll = lnS - dot - b_tgt summed over all tokens
                nllm = capool.tile([128, NTT], F32)
                tmp = capool.tile([128, NTT], F32)
                nc.vector.tensor_tensor(out=tmp[:], in0=dots[:], in1=bts[:], op=ALU.add)
                nc.vector.tensor_tensor(out=nllm[:], in0=lns[:], in1=tmp[:], op=ALU.subtract)
                nsum = capool.tile([128, 1], F32)
                nc.vector.tensor_reduce(out=nsum[:], in_=nllm[:], axis=AX.X, op=ALU.add)
                with tc.tile_pool(name="fin", bufs=1, space=bass.MemorySpace.PSUM) as finpool:
                    ps1 = finpool.tile([1, 1], F32)
                    nc.tensor.matmul(ps1[0:1, 0:1], nsum[:, 0:1], ones_col[:, 0:1],
                                     start=True, stop=True)
                    osb = capool.tile([1, 1], F32)
                    nc.vector.tensor_copy(osb[0:1, :], ps1[0:1, :])
                    nc.sync.dma_start(nll_out[:], osb[0:1, :])

    _split_multiwaits(nc)
    return nc


def _get_program(flags):
    if flags not in _CACHE:
        _CACHE[flags] = _build_program(*flags)
    return _CACHE[flags]


def _prep_host(input_data, targets, embedding, W0, b0, W1, b1, softmax_w, softmax_b):
    """Host-side layout prep (dtype casts, permutes, per-core index arrays)."""
    W0p = np.ascontiguousarray(W0[:, GPERM], dtype=np.float32)
    W1p = np.ascontiguousarray(W1[:, GPERM], dtype=np.float32)
    b0p = np.asarray(b0, np.float32)[GPERM].copy()
    b1p = np.asarray(b1, np.float32)[GPERM].copy()
    b0_nonzero = bool(np.any(b0p))
    b1_nonzero = bool(np.any(b1p))
    smb = np.asarray(softmax_b, np.float32)
    smb_nonzero = bool(np.any(smb))
    # effective bias rows (forget bias folded in)
    b0e = b0p.copy()
    b0e[_F] += FORGET_BIAS
    b1e = b1p.copy()
    b1e[_F] += FORGET_BIAS
    # [V, 264]: softmax_w column + bias + pad
    wvb = np.zeros((V, 264), np.float32)
    wvb[:, 0:U] = np.asarray(softmax_w, np.float32).T
    wvb[:, U] = smb

    shared = {
        "emb": np.ascontiguousarray(embedding, np.float32),
        "w0": W0p,
        "w1": W1p,
        "b0v": b0e[None, :],
        "b1v": b1e[None, :],
        "smw": np.ascontiguousarray(softmax_w, np.float32),
        "smb": smb[None, :],
        "wvb": wvb,
    }
    in_maps = []
    ids = np.asarray(input_data, np.int32)
    tgs = np.asarray(targets, np.int32)
    for c in range(NCORES):
        tok_e = ids[c * BL:(c + 1) * BL, :].T.reshape(-1)   # t-major [1024]
        tok_t = tgs[c * BL:(c + 1) * BL, :].T.reshape(-1)
        m = dict(shared)
        m["idx_e"] = np.ascontiguousarray(tok_e.reshape(NG, 128).T)
        m["idx_t"] = np.ascontiguousarray(tok_t.reshape(NTT, 128).T)
        in_maps.append(m)
    return (b0_nonzero, b1_nonzero, smb_nonzero), in_maps


def run(trace=False, **inputs):
    flags, in_maps = _prep_host(**inputs)
    nc = _get_program(flags)
    res = run_bass_kernel_spmd(nc, in_maps, list(range(NCORES)), trace=trace)
    total = sum(float(r["nll_out"][0, 0]) for r in res.results)
    cost = np.float32(total / (B * T))
    return cost, res


def kernel(**inputs):
    cost, _ = run(trace=False, **inputs)
    return cost


# revision 20
# speedup vs baseline: 1.8210x; 1.8210x over previous
"""CharRNN (2-layer LSTM + softmax CE) Trainium2 Bass kernel.

Sharding: data-parallel over batch (B=64 -> 8 rows/core on 8 cores).
Each core runs the full T=128 recurrence for its 8 sequences and the
cross-entropy over its own 1024 tokens; host sums the 8 partial NLLs.

Device layout (per core):
  - Cell math in batch-on-partitions layout: cell0(t) occupies PSUM/SBUF
    partitions 0:8, cell1(t-1) occupies partitions 32:40 (distinct PE
    column-groups so their matmuls can overlap on the array).
  - z = [x,h] @ W computed with x_h^T as the stationary operand (lhsT) and
    W streamed as the moving operand (weights stream at 1 col/cycle; this
    is the cheap direction for tiny batch).
  - h is transposed each step via TensorE transpose; h1^T columns
    accumulate into OUT^T which later feeds the logits matmul as lhsT.
  - CE: logits tile [128 tokens, 500 vocab] per matmul pair, Exp with
    fused accumulate (sum over vocab chunk), no max-shift (logits are
    O(+-6) for this model), target logit via indirect row gather of the
    host-transposed softmax weights and a fused multiply-reduce.
"""

import sys

for _p in ("/opt/trn_rl_repo",):
    if _p not in sys.path:
        sys.path.insert(0, _p)

import ml_dtypes
import numpy as np

import concourse.bass as bass
import concourse.mybir as mybir
import concourse.tile as tile
from concourse.bass_utils import run_bass_kernel_spmd
from concourse.masks import make_identity

F32 = mybir.dt.float32
BF16 = mybir.dt.bfloat16
I32 = mybir.dt.int32
AF = mybir.ActivationFunctionType
ALU = mybir.AluOpType
AX = mybir.AxisListType

# Problem shapes (hardcoded per contest rules).
V, B, T, U = 16000, 64, 128, 256
NCORES = 8
BL = B // NCORES            # 8 batch rows per core
NTOK = BL * T               # 1024 tokens per core
G4 = 4 * U                  # 1024 gate width
NG = 8                      # embedding gathers per core (128 tokens each)
VC = 500                    # vocab chunk for CE
NVC = V // VC               # 32 chunks
NTT = NTOK // 128           # 8 token tiles for CE

# Gate permutation: reference z columns are [i|j|f|o]; we reorder to
# [i|o|f|j] so sigmoid covers one contiguous [0:768] span and tanh(j) the
# tail. Column ranges in permuted space:
_I = slice(0 * U, 1 * U)    # sigmoid (input gate)
_O = slice(1 * U, 2 * U)    # sigmoid (output gate)
_F = slice(2 * U, 3 * U)    # sigmoid (forget gate, +1.0 bias)
_J = slice(3 * U, 4 * U)    # tanh    (cell candidate)
GPERM = np.r_[0:U, 3 * U:4 * U, 2 * U:3 * U, U:2 * U]

FORGET_BIAS = 1.0

_CACHE = {}


def _split_multiwaits(nc):
    """Walrus (CoreV3 codegen) supports only ONE semaphore wait per HW
    instruction (NEURON_ISA_TPB_EVENTS has a single wait slot) and errors
    out on instructions carrying more. Tile emits multi-wait sync_info
    freely, so split: for an instruction with k>1 waits, insert k-1 NoOps
    on the same engine queue immediately before it, each carrying one
    wait. Identical semantics (the queue processes waits in order)."""
    cnt = 0
    for fn in nc.m.functions:
        for b in fn.blocks:
            live = b.instructions
            out = []
            changed = False
            for i in live:
                si = getattr(i, "sync_info", None)
                waits = list(si.on_wait) if si is not None else []
                movable_idx = [
                    k for k, w in enumerate(waits)
                    if getattr(w, "wait_reg", None) is None
                ]
                if len(waits) > 1 and len(movable_idx) >= 1:
                    # keep the last movable wait (plus any reg-waits) on the
                    # original; hoist the rest onto NoOps.
                    hoist = movable_idx[:-1] if len(movable_idx) == len(waits) \
                        else movable_idx
                    hoist_set = set(hoist)
                    if len(waits) - len(hoist_set) > 1:
                        # still >1 left (multiple reg waits?) — hoist all movable
                        hoist_set = set(movable_idx)
                    for k in sorted(hoist_set):
                        nop = mybir.InstNoOp(name=f"I-nopw{cnt}", ins=[], outs=[])
                        cnt += 1
                        nop.engine = i.engine
                        nop.sync_info = mybir.SyncInfo(
                            on_wait=[waits[k]], on_update=[])
                        out.append(nop)
                    keep = [w for k, w in enumerate(waits) if k not in hoist_set]
                    i.sync_info = mybir.SyncInfo(
                        on_wait=keep, on_update=list(si.on_update))
                    changed = True
                out.append(i)
            if changed:
                live.clear()
                live.extend(out)
    return cnt


def _build_program(b0_nonzero: bool, b1_nonzero: bool, smb_nonzero: bool, debug: bool = False):
    nc = bass.Bass()

    emb = nc.declare_dram_parameter("emb", [V, U], BF16, isOutput=False)
    w0 = nc.declare_dram_parameter("w0", [2 * U, G4], BF16, isOutput=False)
    w1 = nc.declare_dram_parameter("w1", [2 * U, G4], BF16, isOutput=False)
    b0v = nc.declare_dram_parameter("b0v", [1, G4], BF16, isOutput=False)
    b1v = nc.declare_dram_parameter("b1v", [1, G4], BF16, isOutput=False)
    smw = nc.declare_dram_parameter("smw", [U, V], BF16, isOutput=False)
    smb = nc.declare_dram_parameter("smb", [1, V], BF16, isOutput=False)
    wvb = nc.declare_dram_parameter("wvb", [V, 264], F32, isOutput=False)
    idx_e = nc.declare_dram_parameter("idx_e", [128, NG], I32, isOutput=False)
    idx_t = nc.declare_dram_parameter("idx_t", [128, NTT], I32, isOutput=False)
    nll_out = nc.declare_dram_parameter("nll_out", [1, 1], F32, isOutput=True)
    if debug:
        d_outt = nc.declare_dram_parameter("d_outt", [128, 2 * NTOK], BF16, isOutput=True)
        d_sums = nc.declare_dram_parameter("d_sums", [128, NTT * NVC], F32, isOutput=True)
        d_ld = nc.declare_dram_parameter("d_ld", [128, 3 * NTT], F32, isOutput=True)

    with tile.TileContext(nc) as tc:
        with (
            tc.tile_pool(name="const", bufs=1) as cpool,
            tc.tile_pool(name="wpool", bufs=1) as wpool,
            tc.tile_pool(name="persist", bufs=1) as ppool,
        ):
            ident = cpool.tile([128, 128], BF16)
            make_identity(nc, ident[:])
            ones_row = cpool.tile([1, 128], BF16)
            nc.gpsimd.memset(ones_row[:], 1.0)
            ones_col = cpool.tile([128, 1], F32)
            nc.gpsimd.memset(ones_col[:], 1.0)

            # Weights resident in SBUF as 4 k-tiles each: [128, 4*1024].
            w0t = wpool.tile([128, 4, G4], BF16)
            w1t = wpool.tile([128, 4, G4], BF16)
            for k in range(4):
                nc.sync.dma_start(w0t[:, k, :], w0[k * 128:(k + 1) * 128, :])
                nc.sync.dma_start(w1t[:, k, :], w1[k * 128:(k + 1) * 128, :])
            b0t = cpool.tile([1, G4], BF16)
            b1t = cpool.tile([1, G4], BF16)
            nc.sync.dma_start(b0t[:], b0v[:])
            nc.sync.dma_start(b1t[:], b1v[:])
            if smb_nonzero:
                smbt = cpool.tile([1, V], BF16)
                nc.sync.dma_start(smbt[:], smb[:])

            idx_et = cpool.tile([128, NG], I32)
            idx_tt = cpool.tile([128, NTT], I32)
            nc.sync.dma_start(idx_et[:], idx_e[:])
            nc.sync.dma_start(idx_tt[:], idx_t[:])

            # OUT^T: h1 transposed, u on partitions (2 half-tiles), token cols.
            outt = ppool.tile([128, 2, NTOK], BF16)

            # ---------------- embedding gather + transpose ----------------
            xts = []
            with tc.tile_pool(name="xt", bufs=NG) as xtpool:
                with (
                    tc.tile_pool(name="xg", bufs=3) as xgpool,
                    tc.tile_pool(name="ptx", bufs=2, space=bass.MemorySpace.PSUM) as ptxpool,
                ):
                    for g in range(NG):
                        xg = xgpool.tile([128, U], BF16, tag="xg")
                        nc.gpsimd.indirect_dma_start(
                            out=xg[:],
                            out_offset=None,
                            in_=emb[:],
                            in_offset=bass.IndirectOffsetOnAxis(ap=idx_et[:, g:g + 1], axis=0),
                        )
                        xt = xtpool.tile([128, 2, 128], BF16, tag="xt")
                        xts.append(xt)
                        for u in range(2):
                            ptx = ptxpool.tile([128, 128], F32, tag="ptx")
                            nc.tensor.matmul(ptx[:], xg[:, u * 128:(u + 1) * 128], ident[:], start=True, stop=True)
                            nc.vector.tensor_copy(xt[:, u, :], ptx[:])

                # ---------------- LSTM recurrence ----------------
                with (
                    tc.tile_pool(name="zpsum", bufs=2, space=bass.MemorySpace.PSUM) as zpool,
                    tc.tile_pool(name="ptp", bufs=2, space=bass.MemorySpace.PSUM) as ptpool,
                    tc.tile_pool(name="gates", bufs=2) as gpool,
                    tc.tile_pool(name="state", bufs=2) as spool,
                    tc.tile_pool(name="hT", bufs=3) as htpool,
                ):
                    c_prev = None
                    h0T_last = None   # h0^T(s-1): [128, 16] (u-halves x batch)

                    # Sanitize junk: the cell math only writes partition rows
                    # 0:40; the h transposes contract over all 128 partitions,
                    # and NaN junk * identity-zeros = NaN. Zero the unused
                    # rows of both z PSUM slots and both h SBUF slots once.
                    for d in range(2):
                        zz = zpool.tile([128, G4], F32, tag="z", name="zz")
                        nc.scalar.mul(zz[:, :], zz[:, :], 0.0)
                        hz = gpool.tile([128, U], BF16, tag="h", name="hz")
                        nc.gpsimd.memset(hz[:], 0.0)
                        tcz = gpool.tile([128, U], F32, tag="tc", name="tcz")
                        nc.gpsimd.memset(tcz[:], 0.0)
                        gz = gpool.tile([128, 3 * U], F32, tag="G", name="gz")
                        nc.gpsimd.memset(gz[:], 0.0)
                        cz = spool.tile([128, U], F32, tag="c", name="cz")
                        nc.gpsimd.memset(cz[:], 0.0)

                    def emit_group(mms, tp):
                        for i, (o, l, r) in enumerate(mms):
                            nc.tensor.matmul(
                                o, l, r,
                                start=(i == 0), stop=(i == len(mms) - 1),
                                tile_position=tp)

                    for s in range(T + 1):
                        t0 = s            # cell0 step index
                        t1 = s - 1        # cell1 step index
                        cell0 = t0 < T
                        cell1 = 0 <= t1 < T

                        z = zpool.tile([128, G4], F32, tag="z")

                        if cell0:
                            g = t0 // 16
                            c0 = (t0 % 16) * BL
                            xt = xts[g]
                            for n in range(2):
                                ns = slice(n * 512, (n + 1) * 512)
                                mms = [
                                    (z[0:BL, ns], xt[:, 0, c0:c0 + BL], w0t[:, 0, ns]),
                                    (z[0:BL, ns], xt[:, 1, c0:c0 + BL], w0t[:, 1, ns]),
                                ]
                                if b0_nonzero:
                                    mms.append((z[0:BL, ns], ones_row[0:1, 0:BL], b0t[0:1, ns]))
                                elif n == 1:
                                    bs = slice(512, 768)
                                    mms.append((z[0:BL, bs], ones_row[0:1, 0:BL], b0t[0:1, bs]))
                                if t0 >= 1:
                                    mms.append((z[0:BL, ns], h0T_last[:, 0:BL], w0t[:, 2, ns]))
                                    mms.append((z[0:BL, ns], h0T_last[:, BL:2 * BL], w0t[:, 3, ns]))
                                emit_group(mms, None)

                        if cell1:
                            for n in range(2):
                                ns = slice(n * 512, (n + 1) * 512)
                                mms = [
                                    (z[32:32 + BL, ns], h0T_last[:, 0:BL], w1t[:, 0, ns]),
                                    (z[32:32 + BL, ns], h0T_last[:, BL:2 * BL], w1t[:, 1, ns]),
                                ]
                                if b1_nonzero:
                                    mms.append((z[32:32 + BL, ns], ones_row[0:1, 0:BL], b1t[0:1, ns]))
                                elif n == 1:
                                    bs = slice(512, 768)
                                    mms.append((z[32:32 + BL, bs], ones_row[0:1, 0:BL], b1t[0:1, bs]))
                                if t1 >= 1:
                                    mms.append((z[32:32 + BL, ns],
                                                outt[:, 0, (t1 - 1) * BL:t1 * BL], w1t[:, 2, ns]))
                                    mms.append((z[32:32 + BL, ns],
                                                outt[:, 1, (t1 - 1) * BL:t1 * BL], w1t[:, 3, ns]))
                                emit_group(mms, (0, 32))

                        # partition range for the elementwise phase
                        if cell0 and cell1:
                            lo, hi = 0, 40
                        elif cell0:
                            lo, hi = 0, BL
                        else:
                            lo, hi = 32, 40

                        gt = gpool.tile([128, 3 * U], F32, tag="G")
                        tj = gpool.tile([128, U], F32, tag="tj")
                        nc.scalar.activation(gt[lo:hi, :], z[lo:hi, 0:3 * U], AF.Sigmoid)
                        nc.scalar.activation(tj[lo:hi, :], z[lo:hi, 3 * U:G4], AF.Tanh)

                        c_new = spool.tile([128, U], F32, tag="c")
                        if s == 0:
                            # c_new = sigmoid(i) * tanh(j); zero cell1 rows so the
                            # s=1 packed update sees c1(-1) = 0.
                            nc.vector.tensor_tensor(
                                out=c_new[lo:hi, :], in0=gt[lo:hi, _I], in1=tj[lo:hi, :],
                                op=ALU.mult)
                            nc.gpsimd.memset(c_new[32:40, :], 0.0)
                        else:
                            m1 = gpool.tile([128, U], F32, tag="m1")
                            m2 = gpool.tile([128, U], F32, tag="m2")
                            nc.vector.tensor_tensor(
                                out=m1[lo:hi, :], in0=gt[lo:hi, _F], in1=c_prev[lo:hi, :],
                                op=ALU.mult)
                            nc.vector.tensor_tensor(
                                out=m2[lo:hi, :], in0=gt[lo:hi, _I], in1=tj[lo:hi, :],
                                op=ALU.mult)
                            nc.vector.tensor_tensor(
                                out=c_new[lo:hi, :], in0=m1[lo:hi, :], in1=m2[lo:hi, :],
                                op=ALU.add)

                        tc_t = gpool.tile([128, U], F32, tag="tc")
                        h_t = gpool.tile([128, U], BF16, tag="h")
                        nc.scalar.activation(tc_t[lo:hi, :], c_new[lo:hi, :], AF.Tanh)
                        nc.vector.tensor_tensor(
                            out=h_t[lo:hi, :], in0=tc_t[lo:hi, :], in1=gt[lo:hi, _O],
                            op=ALU.mult)

                        h0T_t = None
                        if cell0:
                            h0T_t = htpool.tile([128, 2 * BL], BF16, tag="h0T", name="h0T_t")
                        for u in range(2):
                            pt = ptpool.tile([128, 128], F32, tag="pt")
                            nc.tensor.matmul(pt[:], h_t[:, u * 128:(u + 1) * 128], ident[:],
                                             start=True, stop=True)
                            if cell0:
                                nc.vector.tensor_copy(h0T_t[:, u * BL:(u + 1) * BL], pt[:, 0:BL])
                            if cell1:
                                nc.vector.tensor_copy(
                                    outt[:, u, t1 * BL:(t1 + 1) * BL], pt[:, 32:40])

                        if cell0:
                            h0T_last = h0T_t
                        c_prev = c_new

            # ---------------- cross-entropy phase ----------------
            with (
                tc.tile_pool(name="ce_w", bufs=3) as cwpool,
                tc.tile_pool(name="ce_ps", bufs=3, space=bass.MemorySpace.PSUM) as lppool,
                tc.tile_pool(name="ce_pt", bufs=2, space=bass.MemorySpace.PSUM) as ptcpool,
                tc.tile_pool(name="ce_sb", bufs=3) as cspool,
                tc.tile_pool(name="ce_acc", bufs=1) as capool,
            ):
                sums = capool.tile([128, NTT, NVC], F32)
                for vc in range(NVC):
                    wt = cwpool.tile([128, 2, VC], BF16, tag="wt")
                    for u in range(2):
                        nc.sync.dma_start(
                            wt[:, u, :], smw[u * 128:(u + 1) * 128, vc * VC:(vc + 1) * VC])
                    for tk in range(NTT):
                        lp = lppool.tile([128, VC], F32, tag="lp")
                        nc.tensor.matmul(
                            lp[:], outt[:, 0, tk * 128:(tk + 1) * 128], wt[:, 0, :],
                            start=True, stop=False)
                        nc.tensor.matmul(
                            lp[:], outt[:, 1, tk * 128:(tk + 1) * 128], wt[:, 1, :],
                            start=False, stop=not smb_nonzero)
                        if smb_nonzero:
                            nc.tensor.matmul(
                                lp[:], ones_row[0:1, :], smbt[0:1, vc * VC:(vc + 1) * VC],
                                start=False, stop=True)
                        es = cspool.tile([128, VC], F32, tag="es")
                        nc.scalar.activation(
                            es[:], lp[:], AF.Exp, accum_out=sums[:, tk, vc:vc + 1])

                lns = capool.tile([128, NTT], F32)
                dots = capool.tile([128, NTT], F32)
                bts = capool.tile([128, NTT], F32)
                for tk in range(NTT):
                    s1 = cspool.tile([128, 1], F32, tag="s1")
                    nc.vector.tensor_reduce(
                        out=s1[:], in_=sums[:, tk, :], axis=AX.X, op=ALU.add)
                    nc.scalar.activation(lns[:, tk:tk + 1], s1[:], AF.Ln)

                    wtg = cspool.tile([128, 264], F32, tag="wtg")
                    nc.gpsimd.indirect_dma_start(
                        out=wtg[:],
                        out_offset=None,
                        in_=wvb[:],
                        in_offset=bass.IndirectOffsetOnAxis(ap=idx_tt[:, tk:tk + 1], axis=0),
                    )
                    outb = cspool.tile([128, U], F32, tag="outb")
                    for u in range(2):
                        ptc = ptcpool.tile([128, 128], F32, tag="ptc")
                        nc.tensor.matmul(
                            ptc[:], outt[:, u, tk * 128:(tk + 1) * 128], ident[:],
                            start=True, stop=True)
                        nc.vector.tensor_copy(outb[:, u * 128:(u + 1) * 128], ptc[:])
                    ttr = cspool.tile([128, U], F32, tag="ttr")
                    nc.vector.tensor_tensor(
                        out=ttr[:], in0=outb[:], in1=wtg[:, 0:U], op=ALU.mult)
                    nc.vector.tensor_reduce(
                        out=dots[:, tk:tk + 1], in_=ttr[:], axis=AX.X, op=ALU.add)
                    nc.vector.tensor_copy(bts[:, tk:tk + 1], wtg[:, U:U + 1])

                # nll = lnS - dot - b_tgt summed over all tokens
                nllm = capool.tile([128, NTT], F32)
                tmp = capool.tile([128, NTT], F32)
                nc.vector.tensor_tensor(out=tmp[:], in0=dots[:], in1=bts[:], op=ALU.add)
                nc.vector.tensor_tensor(out=nllm[:], in0=lns[:], in1=tmp[:], op=ALU.subtract)
                if debug:
                    nc.sync.dma_start(d_outt[:], outt[:].rearrange("p a b -> p (a b)"))
                    nc.sync.dma_start(d_sums[:], sums[:].rearrange("p a b -> p (a b)"))
                    nc.sync.dma_start(d_ld[:, 0:NTT], lns[:])
                    nc.sync.dma_start(d_ld[:, NTT:2 * NTT], dots[:])
                    nc.sync.dma_start(d_ld[:, 2 * NTT:3 * NTT], bts[:])
                nsum = capool.tile([128, 1], F32)
                nc.vector.tensor_reduce(out=nsum[:], in_=nllm[:], axis=AX.X, op=ALU.add)
                with tc.tile_pool(name="fin", bufs=1, space=bass.MemorySpace.PSUM) as finpool:
                    ps1 = finpool.tile([1, 1], F32)
                    nc.tensor.matmul(ps1[0:1, 0:1], nsum[:, 0:1], ones_col[:, 0:1],
                                     start=True, stop=True)
                    osb = capool.tile([1, 1], F32)
                    nc.vector.tensor_copy(osb[0:1, :], ps1[0:1, :])
                    nc.sync.dma_start(nll_out[:], osb[0:1, :])

    _split_multiwaits(nc)
    return nc


def _get_program(flags):
    if flags not in _CACHE:
        _CACHE[flags] = _build_program(*flags)
    return _CACHE[flags]


def _prep_host(input_data, targets, embedding, W0, b0, W1, b1, softmax_w, softmax_b):
    """Host-side layout prep (dtype casts, permutes, per-core index arrays)."""
    W0p = np.ascontiguousarray(W0[:, GPERM], dtype=np.float32)
    W1p = np.ascontiguousarray(W1[:, GPERM], dtype=np.float32)
    b0p = np.asarray(b0, np.float32)[GPERM].copy()
    b1p = np.asarray(b1, np.float32)[GPERM].copy()
    b0_nonzero = bool(np.any(b0p))
    b1_nonzero = bool(np.any(b1p))
    smb = np.asarray(softmax_b, np.float32)
    smb_nonzero = bool(np.any(smb))
    # effective bias rows (forget bias folded in)
    b0e = b0p.copy()
    b0e[_F] += FORGET_BIAS
    b1e = b1p.copy()
    b1e[_F] += FORGET_BIAS
    # [V, 264]: softmax_w column + bias + pad
    wvb = np.zeros((V, 264), np.float32)
    wvb[:, 0:U] = np.asarray(softmax_w, np.float32).T
    wvb[:, U] = smb

    bf = ml_dtypes.bfloat16
    shared = {
        "emb": np.ascontiguousarray(np.asarray(embedding, np.float32)).astype(bf),
        "w0": W0p.astype(bf),
        "w1": W1p.astype(bf),
        "b0v": b0e[None, :].astype(bf),
        "b1v": b1e[None, :].astype(bf),
        "smw": np.ascontiguousarray(np.asarray(softmax_w, np.float32)).astype(bf),
        "smb": smb[None, :].astype(bf),
        "wvb": wvb,
    }
    in_maps = []
    ids = np.asarray(input_data, np.int32)
    tgs = np.asarray(targets, np.int32)
    for c in range(NCORES):
        tok_e = ids[c * BL:(c + 1) * BL, :].T.reshape(-1)   # t-major [1024]
        tok_t = tgs[c * BL:(c + 1) * BL, :].T.reshape(-1)
        m = dict(shared)
        m["idx_e"] = np.ascontiguousarray(tok_e.reshape(NG, 128).T)
        m["idx_t"] = np.ascontiguousarray(tok_t.reshape(NTT, 128).T)
        in_maps.append(m)
    return (b0_nonzero, b1_nonzero, smb_nonzero), in_maps


def run(trace=False, **inputs):
    flags, in_maps = _prep_host(**inputs)
    nc = _get_program(flags)
    res = run_bass_kernel_spmd(nc, in_maps, list(range(NCORES)), trace=trace)
    total = sum(float(r["nll_out"][0, 0]) for r in res.results)
    cost = np.float32(total / (B * T))
    return cost, res


def kernel(**inputs):
    cost, _ = run(trace=False, **inputs)
    return cost
